# revision 1
# baseline (speedup 1.0000x reference)
"""Trainium2 Bass kernel: out = expm(-t*L) @ x  (graph-Laplacian diffusion).

Sharding: each of 8 cores owns a 32-column feature slab (per the hint) and
runs all 32 Taylor matvecs (4 segments x 8 terms) independently; the edge
structure is replicated.

The Laplacian diagonal is folded into self-loop edges (i, i, -deg_i), so a
matvec is one sparse pass: term_k = (t_seg/k) * (W - D) @ term_{k-1}.
Per 1024-edge chunk (edges globally row-sorted):
  - dma_gather from an HBM node-PAIR table ([25024, 128] fp32, 512B rows):
    gather idx = col//2 stays in int16 range;
  - DVE parity select-and-scale using host-prescaled values (t_seg/k and
    the parity mask folded in; padding slots carry value 0);
  - dma_scatter_add into an HBM accumulator with per-chunk row bases
    (idx = row - base fits int16: a row-sorted chunk spans <= ~1100 rows).

Chunk counts / bases / wrap layouts are compile-time specialized to the
edge structure; index and value arrays are runtime inputs.
"""
import numpy as np

N_NODES = 50000
N_PAD = 50048          # multiple of 128
D_FEAT = 256
DIFF_T = 1.0
N_SEG = 4
N_TERMS = 8
NCORES = 8
SLAB = D_FEAT // NCORES          # 32
CHUNK = 1024
DVE_GROUP = 2
NROW_SP = N_PAD // 128           # 391
QROWS = 98                       # row-blocks per update quarter (4*98>=391)

_compiled = {}


def _wrap_idx(idx):
    """[1024] int16 -> [128, 64]: idx j at (j%16, j//16), replicated x8."""
    return np.tile(idx.reshape(-1, 16).T, (8, 1)).astype(np.int16)


def _preprocess(edge_rows, edge_cols, edge_vals):
    er = np.asarray(edge_rows).astype(np.int64)
    ec = np.asarray(edge_cols).astype(np.int64)
    ev = np.asarray(edge_vals).astype(np.float64)

    deg = np.zeros(N_NODES, np.float64)
    np.add.at(deg, er, ev)

    rows = np.concatenate([er, np.arange(N_NODES)])
    cols = np.concatenate([ec, np.arange(N_NODES)])
    ws = np.concatenate([ev, -deg])

    order = np.argsort(rows, kind="stable")
    rows, cols, ws = rows[order], cols[order], ws[order]

    # occurrence index of each edge within its row -> "round"; a round
    # touches each row at most once, so the scatter-adds of one round are
    # duplicate-free (the HW CCE read-modify-write races on duplicates).
    e_all = rows.shape[0]
    first = np.r_[True, rows[1:] != rows[:-1]]
    run_start = np.where(first, np.arange(e_all), 0)
    run_start = np.maximum.accumulate(run_start)
    occ = np.arange(e_all) - run_start

    order2 = np.lexsort((rows, occ))
    rows, cols, ws, occ = rows[order2], cols[order2], ws[order2], occ[order2]

    # Build 1024-edge chunks: never straddle a round boundary, keep each
    # chunk's row span < 32767 (int16 scatter offsets), and pad each round
    # to an even chunk count so DVE groups stay within one round.
    SPAN = 32000
    r_list, c_list, w_list, round_of_chunk = [], [], [], []
    n_rounds = int(occ.max()) + 1
    for r in range(n_rounds):
        m = occ == r
        rr, cc, ww = rows[m], cols[m], ws[m]
        chunks = []
        i = 0
        while i < len(rr):
            j = i
            while (j < len(rr) and j - i < CHUNK
                   and rr[j] - rr[i] < SPAN):
                j += 1
            chunks.append((i, j))
            i = j
        if len(chunks) % DVE_GROUP:
            chunks.append((len(rr), len(rr)))  # empty pad chunk
        # rows present in this round (pad targets must avoid them: the HW
        # CCE scatter-add races on duplicate rows within a fence window)
        present = set(rr.tolist())
        for (i, j) in chunks:
            padn = CHUNK - (j - i)
            if padn:
                base_row = int(rr[i]) if j > i else int(rr[-1])
                dump = None
                for cand in range(base_row, min(base_row + 31000, N_PAD)):
                    if cand not in present:
                        dump = cand
                        break
                if dump is None:  # dense region: use the pad rows >= N_NODES
                    dump = N_PAD - 1
                    assert dump - base_row < 32000
                prow = dump
            else:
                prow = 0
            r_list.append(np.concatenate([rr[i:j], np.full(padn, prow)]))
            c_list.append(np.concatenate([cc[i:j], np.zeros(padn, np.int64)]))
            w_list.append(np.concatenate([ww[i:j], np.zeros(padn)]))
            round_of_chunk.append(r)
    rows = np.concatenate(r_list)
    cols = np.concatenate(c_list)
    ws = np.concatenate(w_list)
    n_chunks = len(rows) // CHUNK

    wcols = CHUNK // 16
    bases = []
    gidx_w = np.empty((128, n_chunks * wcols), np.int16)
    sidx_w = np.empty((128, n_chunks * wcols), np.int16)
    valA = np.empty((128, n_chunks * 8), np.float64)
    valB = np.empty((128, n_chunks * 8), np.float64)
    for c in range(n_chunks):
        sl = slice(c * CHUNK, (c + 1) * CHUNK)
        base = int(rows[sl][0])
        bases.append(base)
        srel = rows[sl] - base
        assert 0 <= srel.min() and srel.max() < 32767
        gidx_w[:, c * wcols:(c + 1) * wcols] = _wrap_idx(
            (cols[sl] // 2).astype(np.int16))
        sidx_w[:, c * wcols:(c + 1) * wcols] = _wrap_idx(
            srel.astype(np.int16))
        par = (cols[sl] % 2).astype(np.float64)
        valA[:, c * 8:(c + 1) * 8] = (ws[sl] * (1.0 - par)).reshape(8, 128).T
        valB[:, c * 8:(c + 1) * 8] = (ws[sl] * par).reshape(8, 128).T

    t_seg = DIFF_T / N_SEG
    vals = np.empty((N_TERMS, 128, 2 * n_chunks * 8), np.float32)
    for ki in range(N_TERMS):
        s = t_seg / (ki + 1)
        vals[ki, :, 0::2] = (valA * s).astype(np.float32)
        vals[ki, :, 1::2] = (valB * s).astype(np.float32)

    return n_chunks, bases, round_of_chunk, gidx_w, sidx_w, vals


def _build(n_chunks, bases, round_of_chunk, n_seg):
    import concourse.bacc as bacc
    import concourse.mybir as mybir
    from concourse.bass import AP

    NG = n_chunks // DVE_GROUP
    WC = CHUNK // 16
    GS = DVE_GROUP * 8            # 16 slots per DVE group

    nc = bacc.Bacc("TRN2", debug=False, num_devices=NCORES,
                   num_swdge_queues=2)
    x_in = nc.dram_tensor("x_slab", [N_PAD, 64], mybir.dt.float32,
                          kind="ExternalInput").ap()
    gidx_in = nc.dram_tensor("gidx", [128, n_chunks * WC], mybir.dt.int16,
                             kind="ExternalInput").ap()
    sidx_in = nc.dram_tensor("sidx", [128, n_chunks * WC], mybir.dt.int16,
                             kind="ExternalInput").ap()
    vals_in = nc.dram_tensor("vals", [N_TERMS, 128, 2 * n_chunks * 8],
                             mybir.dt.float32, kind="ExternalInput").ap()
    out_t = nc.dram_tensor("out", [N_PAD, SLAB], mybir.dt.float32,
                           kind="ExternalOutput").ap()
    vtab = nc.dram_tensor("vtab", [N_PAD, 64], mybir.dt.float32,
                          kind="Internal").ap()
    agg = nc.dram_tensor("agg", [N_PAD, 64], mybir.dt.float32,
                         kind="Internal").ap()
    acch = nc.dram_tensor("acch", [N_PAD, SLAB], mybir.dt.float32,
                          kind="Internal").ap()

    v2 = vtab.rearrange("(a b) f -> a (b f)", b=2)               # [25024,128]
    agg32 = agg[:, 0:SLAB].rearrange("(n p) f -> p n f", p=128)  # node=n*128+p
    x32 = x_in[:, 0:SLAB].rearrange("(n p) f -> p n f", p=128)
    vtab32 = vtab[:, 0:SLAB].rearrange("(n p) f -> p n f", p=128)
    acc32 = acch.rearrange("(n p) f -> p n f", p=128)
    out32 = out_t.rearrange("(n p) f -> p n f", p=128)
    aggflat = agg.rearrange("(g r) f -> g (r f)", g=16)          # [16, 200192]

    with (
        nc.sbuf_tensor([128, n_chunks * WC], mybir.dt.int16) as gidx,
        nc.sbuf_tensor([128, n_chunks * WC], mybir.dt.int16) as sidx,
        nc.sbuf_tensor([128, 2 * n_chunks * 8], mybir.dt.float32) as vsl,
        nc.sbuf_tensor([128, 2 * GS * 128], mybir.dt.float32) as gbuf,
        nc.sbuf_tensor([128, 2 * GS * 64], mybir.dt.float32) as wvb,
        nc.sbuf_tensor([128, GS * 32], mybir.dt.float32) as tmp,
        nc.sbuf_tensor([128, QROWS * SLAB], mybir.dt.float32) as ubuf,
        nc.sbuf_tensor([128, QROWS * SLAB], mybir.dt.float32) as abuf,
        nc.sbuf_tensor([128, 200192 // 128], mybir.dt.float32) as zbuf,
        nc.semaphore() as usem,
        nc.semaphore() as zsem,
        nc.semaphore() as vsem,
        nc.Block() as block,
    ):
        gsem4 = [[[nc.alloc_semaphore(name=f"gs{i}_{q}_{p}") for p in range(2)]
                  for q in range(2)] for i in range(N_TERMS)]
        ssem4 = [[[nc.alloc_semaphore(name=f"ss{i}_{q}_{p}") for p in range(2)]
                  for q in range(2)] for i in range(N_TERMS)]
        zagg = [nc.alloc_semaphore(name=f"zagg{i}") for i in range(N_TERMS)]
        msem = [nc.alloc_semaphore(name=f"msem{i}") for i in range(N_TERMS)]
        rsem = [nc.alloc_semaphore(name=f"rsem{i}") for i in range(N_TERMS)]
        m2sem = [nc.alloc_semaphore(name=f"m2sem{i}") for i in range(N_TERMS)]

        gb = gbuf[:].rearrange("p (b s e) -> p b s e", b=2, e=128)
        wv = wvb[:].rearrange("p (b s e) -> p b s e", b=2, e=64)
        tp = tmp[:].rearrange("p (s e) -> p s e", e=32)
        ub = ubuf[:].rearrange("p (n f) -> p n f", f=SLAB)
        ab = abuf[:].rearrange("p (n f) -> p n f", f=SLAB)

        def val_ap(g, which):
            t = vsl[:, 2 * g * GS + which: 2 * (g + 1) * GS]
            return AP(t.tensor, t.offset, [t.ap[0], [2, GS], [0, 32]])

        def qsl(q):
            return slice(q * QROWS, min((q + 1) * QROWS, NROW_SP))

        @block.gpsimd
        def _(gpsimd):
            U = [0]      # usem running total
            G4 = [[[0, 0], [0, 0]] for _ in range(N_TERMS)]
            S4 = [[[0, 0], [0, 0]] for _ in range(N_TERMS)]
            Z = [0] * N_TERMS
            M = [0] * N_TERMS
            R = [0] * N_TERMS
            M2 = [0] * N_TERMS

            def u_inc():
                U[0] += 16
                return U[0]

            gpsimd.dma_start(gidx[:], gidx_in[:]).then_inc(usem, 16); u_inc()
            gpsimd.dma_start(sidx[:], sidx_in[:]).then_inc(usem, 16); u_inc()
            gpsimd.dma_start(vtab[:, :], x_in[:, :]).then_inc(usem, 16); u_inc()
            for q in range(4):
                gpsimd.dma_start(acc32[:, qsl(q), :],
                                 x32[:, qsl(q), :]).then_inc(usem, 16); u_inc()
            gpsimd.memzero(zbuf[:]).then_inc(zsem, 1)
            gpsimd.memzero(wvb[:]).then_inc(zsem, 1)
            gpsimd.wait_ge(usem, U[0])
            gpsimd.wait_ge(zsem, 2)

            for seg in range(n_seg):
                for q in range(4):
                    gpsimd.dma_start(vtab32[:, qsl(q), :],
                                     acc32[:, qsl(q), :]).then_inc(usem, 16); u_inc()
                gpsimd.wait_ge(usem, U[0])

                for k in range(N_TERMS):
                    gpsimd.dma_start(vsl[:], vals_in[k]).then_inc(
                        vsem, 16)
                    for q in range(16):
                        gpsimd.dma_start(
                            aggflat[q].rearrange("(p f) -> p f", p=128),
                            zbuf[:, :],
                        ).then_inc(zagg[k], 16)
                        Z[k] += 16
                    zero_done = Z[k]

                    for g in range(NG + 1):
                        if g < NG:
                            for j in range(DVE_GROUP):
                                c = g * DVE_GROUP + j
                                qn, par = c % 2, g % 2
                                gpsimd.dma_gather(
                                    out_ap=gb[:, g % 2, j * 8:(j + 1) * 8, :],
                                    in_ap=v2[:, :],
                                    idxs_ap=gidx[:, c * WC:(c + 1) * WC],
                                    num_idxs=CHUNK, num_idxs_reg=CHUNK,
                                    elem_size=128,
                                    queue_num=qn,
                                ).then_inc(gsem4[k][qn][par], 16)
                                G4[k][qn][par] += 16
                        if g == 0:
                            gpsimd.wait_ge(zagg[k], zero_done)
                        if g > 0:
                            gp = g - 1
                            c0 = gp * DVE_GROUP
                            if c0 > 0 and (round_of_chunk[c0] !=
                                           round_of_chunk[c0 - 1]):
                                for qn in range(2):
                                    for par in range(2):
                                        gpsimd.wait_ge(ssem4[k][qn][par],
                                                       S4[k][qn][par])
                            gpsimd.wait_ge(msem[k], M[k] + gp + 1)
                            for j in range(DVE_GROUP):
                                c = gp * DVE_GROUP + j
                                base = bases[c]
                                span = min(32768, N_PAD - base)
                                qn, par = c % 2, gp % 2
                                gpsimd.dma_scatter_add(
                                    out_ap=agg[base:base + span, :],
                                    in_ap=wv[:, gp % 2, j * 8:(j + 1) * 8, :],
                                    idxs_ap=sidx[:, c * WC:(c + 1) * WC],
                                    num_idxs=CHUNK, num_idxs_reg=CHUNK,
                                    elem_size=64,
                                    queue_num=qn,
                                ).then_inc(ssem4[k][qn][par], 16)
                                S4[k][qn][par] += 16
                    M[k] += NG

                    for qn in range(2):
                        for par in range(2):
                            gpsimd.wait_ge(gsem4[k][qn][par], G4[k][qn][par])
                    gpsimd.wait_ge(vsem, 16 * (seg * N_TERMS + k + 1))
                    for qn in range(2):
                        for par in range(2):
                            gpsimd.wait_ge(ssem4[k][qn][par], S4[k][qn][par])

                    for q in range(4):
                        nr = qsl(q).stop - q * QROWS
                        gpsimd.dma_start(ub[:, 0:nr, :],
                                         agg32[:, qsl(q), :]).then_inc(usem, 16); u_inc()
                        gpsimd.dma_start(ab[:, 0:nr, :],
                                         acc32[:, qsl(q), :]).then_inc(usem, 16); u_inc()
                        gpsimd.wait_ge(usem, U[0])
                        gpsimd.engine_nop().then_inc(rsem[k], 1)
                        R[k] += 1
                        gpsimd.wait_ge(m2sem[k], R[k])
                        gpsimd.dma_start(acc32[:, qsl(q), :],
                                         ab[:, 0:nr, :]).then_inc(usem, 16); u_inc()
                        if k < N_TERMS - 1 or seg < n_seg - 1:
                            gpsimd.dma_start(vtab32[:, qsl(q), :],
                                             ub[:, 0:nr, :]).then_inc(usem, 16); u_inc()
                        gpsimd.wait_ge(usem, U[0])

            gpsimd.dma_start(out_t[:, :], acch[:, :]).then_inc(usem, 16); u_inc()
            gpsimd.wait_ge(usem, U[0])

        @block.vector
        def _(vector):
            V4 = [[[0, 0], [0, 0]] for _ in range(N_TERMS)]
            VS4 = [[[0, 0], [0, 0]] for _ in range(N_TERMS)]
            VM = [0] * N_TERMS
            VR = [0] * N_TERMS
            for seg in range(n_seg):
                for k in range(N_TERMS):
                    vector.wait_ge(vsem, 16 * (seg * N_TERMS + k + 1))
                    for g in range(NG):
                        par = g % 2
                        step = 16 * (g // 2 + 1)
                        for qn in range(2):
                            vector.wait_ge(gsem4[k][qn][par],
                                           V4[k][qn][par] + step)
                        if g >= 2:
                            sdone = 16 * (g // 2)
                            for qn in range(2):
                                vector.wait_ge(ssem4[k][qn][par],
                                               VS4[k][qn][par] + sdone)
                        b = g % 2
                        gsl = gb[:, b]
                        wsl = wv[:, b]
                        vector.tensor_mul(tp[:, :, :], gsl[:, :, 0:32],
                                          val_ap(g, 0))
                        vector.tensor_mul(wsl[:, :, 0:32], gsl[:, :, 64:96],
                                          val_ap(g, 1))
                        vector.drain()
                        vector.tensor_add(wsl[:, :, 0:32], wsl[:, :, 0:32],
                                          tp[:, :, :])
                        vector.drain().then_inc(msem[k], 1)
                    for qn in range(2):
                        V4[k][qn][0] += 16 * ((NG + 1) // 2)
                        V4[k][qn][1] += 16 * (NG // 2)
                        VS4[k][qn][0] += 16 * ((NG + 1) // 2)
                        VS4[k][qn][1] += 16 * (NG // 2)
                    VM[k] += NG
                    for q in range(4):
                        nrows = min((q + 1) * QROWS, NROW_SP) - q * QROWS
                        vector.wait_ge(rsem[k], VR[k] + q + 1)
                        vector.tensor_add(ab[:, 0:nrows, :], ab[:, 0:nrows, :],
                                          ub[:, 0:nrows, :])
                        vector.drain().then_inc(m2sem[k], 1)
                    VR[k] += 4

    nc.compile()
    return nc


def _get_compiled(n_chunks, bases, round_of_chunk, n_seg=N_SEG):
    key = (n_chunks, tuple(bases), tuple(round_of_chunk), n_seg)
    if key not in _compiled:
        _compiled[key] = _build(n_chunks, bases, round_of_chunk, n_seg)
    return _compiled[key]


def kernel(x, edge_rows, edge_cols, edge_vals):
    from concourse.bass_utils import run_bass_kernel_spmd

    x = np.asarray(x, dtype=np.float32)
    n_chunks, bases, round_of_chunk, gidx_w, sidx_w, vals = _preprocess(
        edge_rows, edge_cols, edge_vals)
    nc = _get_compiled(n_chunks, bases, round_of_chunk)

    in_maps = []
    for c in range(NCORES):
        xs = np.zeros((N_PAD, 64), np.float32)
        xs[:N_NODES, 0:SLAB] = x[:, c * SLAB:(c + 1) * SLAB]
        in_maps.append({
            "x_slab": xs, "gidx": gidx_w, "sidx": sidx_w, "vals": vals,
        })
    res = run_bass_kernel_spmd(nc, in_maps, core_ids=list(range(NCORES)))
    out = np.empty((N_NODES, D_FEAT), np.float32)
    for c in range(NCORES):
        out[:, c * SLAB:(c + 1) * SLAB] = res.results[c]["out"][:N_NODES]
    return out



# revision 19
# speedup vs baseline: 51.1875x; 51.1875x over previous
"""Trainium2 Bass kernel: out = expm(-t*L) @ x  (graph-Laplacian diffusion).

Design (driven by measured axon-TRN2 cost structure: ~25-60us per executed
instruction, globally serialized across engines; dma_gather limited to 1024
idxs/op; descriptors/bytes comparatively cheap; 8-core DRAM AllGather works):

- Shard EDGES by destination row across 8 cores (6272-row regions), full
  256-feature width. Each core keeps a replicated fp16 node-pair table
  [25088, 512] of the current Taylor term in its own HBM; per term it
  gathers its ELL slots (1KB pair rows, idx=col//2 fits int16), does a
  fused DVE multiply (weight x parity-select) + segmented reduce over ELL
  slots, scales by t_seg/k, accumulates, and AllGathers the new term's
  regions back into both cores' tables (double-buffered across terms).
- Rows are degree-sorted globally and dealt round-robin to regions, so the
  per-128-row-block ELL widths S_b are near-identical across cores; the
  compiled program (one SPMD module for all 8 cores) uses the max over
  cores, keeping padding ~few %.
- Self-loops (i, i, -deg_i) fold the Laplacian diagonal into the same
  gather/reduce stream; the Taylor recurrence v_k = (t_seg/k)(W-D)v_{k-1},
  acc += v_k runs 4 segments x 8 terms; at segment ends the table is
  refilled from acc.
"""
import numpy as np

N_NODES = 50000
NP = 50176            # 8 * 6272, multiple of 1024
NCORES = 8
R = NP // NCORES      # 6272 region rows per core
RB = R // 128         # 49 blocks
D_FEAT = 256
DIFF_T = 1.0
N_SEG = 4
N_TERMS = 8
GIDX = 1024           # idxs per dma_gather (hard HW ring limit)
QOP = GIDX // 128     # q-rows per gather op
RAWQ = 96             # max q-rows per DVE batch (SBUF budget)

_compiled = {}


def _wrap_idx(idx):
    """[1024] int16 -> [128, 64]: idx j at (j%16, j//16), replicated x8."""
    return np.tile(idx.reshape(-1, 16).T, (8, 1)).astype(np.int16)


def _preprocess(x, edge_rows, edge_cols, edge_vals):
    er = np.asarray(edge_rows).astype(np.int64)
    ec = np.asarray(edge_cols).astype(np.int64)
    ev = np.asarray(edge_vals).astype(np.float64)

    deg = np.zeros(N_NODES, np.float64)
    np.add.at(deg, er, ev)

    rows = np.concatenate([er, np.arange(N_NODES)])
    cols = np.concatenate([ec, np.arange(N_NODES)])
    ws = np.concatenate([ev, -deg])

    cnt = np.bincount(rows, minlength=NP)
    order = np.argsort(-cnt, kind="stable")          # rank -> orig id
    new_of_orig = np.empty(NP, np.int64)
    ranks = np.arange(NP)
    new_of_orig[order] = (ranks % NCORES) * R + ranks // NCORES

    nr = new_of_orig[rows]
    ncol = new_of_orig[cols]

    # per-block ELL width, uniform across cores (max)
    new_cnt = np.zeros(NP, np.int64)
    new_cnt[nr] += 0
    np.add.at(new_cnt, nr, 1)
    S = np.zeros(RB, np.int64)
    for c in range(NCORES):
        reg = new_cnt[c * R:(c + 1) * R].reshape(RB, 128)
        S = np.maximum(S, reg.max(axis=1))
    S = [int(v) for v in S]

    # batch plan: consecutive nonzero blocks, sum S <= RAWQ, pad to x QOP
    batches = []  # list of (blocks list, batch_q0, Q_padded)
    blk_q0 = np.zeros(RB, np.int64)
    qcur = 0
    cur = []
    cur_q0 = 0

    def close():
        nonlocal qcur, cur
        if not cur:
            return
        qb = qcur - cur_q0
        qpad = (-qb) % QOP
        qcur += qpad
        batches.append((list(cur), int(cur_q0), int(qcur - cur_q0)))
        cur = []

    for b in range(RB):
        if S[b] == 0:
            continue
        if cur and (qcur - cur_q0) + S[b] > RAWQ:
            close()
        if not cur:
            cur_q0 = qcur
        cur.append(b)
        blk_q0[b] = qcur
        qcur += S[b]
    close()
    QTOT = qcur
    n_ops = QTOT // QOP

    # reduce runs per batch: (qloc0, b0, nb, S)
    runs = []
    for (blks, q0, qp) in batches:
        rr_ = []
        i = 0
        while i < len(blks):
            j = i
            while j < len(blks) and S[blks[j]] == S[blks[i]]:
                j += 1
            rr_.append((int(blk_q0[blks[i]] - q0), blks[i], j - i,
                        int(S[blks[i]])))
            i = j
        runs.append(rr_)

    # per-core slot arrays
    core_of = nr // R
    pair_idxs = []
    abs_ = []
    for c in range(NCORES):
        m = core_of == c
        rr_c = nr[m] - c * R
        cc_c = ncol[m]
        w_c = ws[m]
        o2 = np.argsort(rr_c, kind="stable")
        rr_c, cc_c, w_c = rr_c[o2], cc_c[o2], w_c[o2]
        e_all = rr_c.shape[0]
        first = np.r_[True, rr_c[1:] != rr_c[:-1]]
        run_start = np.where(first, np.arange(e_all), 0)
        run_start = np.maximum.accumulate(run_start)
        occ = np.arange(e_all) - run_start

        q_e = blk_q0[rr_c // 128] + occ
        p_e = rr_c % 128
        i_e = q_e * 128 + p_e

        pidx = np.zeros(QTOT * 128, np.int16)
        ab = np.zeros((QTOT * 128, 2), np.float16)
        pidx[i_e] = (cc_c // 2).astype(np.int16)
        par = (cc_c % 2).astype(np.float64)
        ab[i_e, 0] = (w_c * (1.0 - par)).astype(np.float16)
        ab[i_e, 1] = (w_c * par).astype(np.float16)

        gidx_w = np.empty((128, n_ops * 64), np.int16)
        for o in range(n_ops):
            gidx_w[:, o * 64:(o + 1) * 64] = _wrap_idx(
                pidx[o * GIDX:(o + 1) * GIDX])
        # ab sbuf layout [128, QTOT, 2]: slot (q, p) at [p, q, :]
        ab_sb = ab.reshape(QTOT, 128, 2).transpose(1, 0, 2).reshape(
            128, QTOT * 2).copy()
        pair_idxs.append(gidx_w)
        abs_.append(ab_sb)

    # permuted x regions
    xp = np.zeros((NP, D_FEAT), np.float32)
    xp[new_of_orig[:N_NODES]] = np.asarray(x, np.float32)
    x_regions = [xp[c * R:(c + 1) * R].copy() for c in range(NCORES)]

    plan = {
        "batches": batches, "runs": runs, "QTOT": QTOT, "n_ops": n_ops,
    }
    return plan, pair_idxs, abs_, x_regions, new_of_orig


def _cheb_fit(er, ec, ev, tol=1.2e-3, d_max=20):
    """Fit q(z) ~ [T8(-t_seg z)]^4 on a complex rectangle covering spec(L).

    Returns (coefs_monomial-in-chebbasis, lam_m, degree) or None."""
    er = np.asarray(er).astype(np.int64)
    ec = np.asarray(ec).astype(np.int64)
    ev = np.asarray(ev).astype(np.float64)
    deg = np.zeros(N_NODES, np.float64)
    np.add.at(deg, er, ev)

    def matvec(v):
        wv = ev * v[ec]
        agg = np.bincount(er, weights=wv, minlength=N_NODES)
        return deg * v - agg

    rng = np.random.default_rng(12345)
    v = rng.standard_normal(N_NODES)
    v /= np.linalg.norm(v)
    lam = 0.0
    for _ in range(60):
        w = matvec(v)
        lam = np.linalg.norm(w)
        v = w / lam
    lam_m = float(lam) * 1.06

    t_seg = DIFF_T / N_SEG

    def pref(z):
        acc = np.ones_like(z)
        term = np.ones_like(z)
        for k in range(1, N_TERMS + 1):
            term = term * (-t_seg * z) / k
            acc = acc + term
        return acc ** N_SEG

    xs = np.linspace(-0.3, lam_m, 500)
    ys = np.linspace(-2.2, 2.2, 61)
    Z = (xs[:, None] + 1j * ys[None, :]).ravel()
    F = pref(Z)
    Zs = Z * (2.0 / lam_m) - 1.0     # scaled: spec -> approx [-1,1]
    for d in range(8, d_max + 1):
        V = np.polynomial.chebyshev.chebvander(Zs.real, d).astype(complex)
        # chebvander on complex: evaluate manually via recurrence
        Vc = np.empty((Z.size, d + 1), complex)
        Vc[:, 0] = 1.0
        if d >= 1:
            Vc[:, 1] = Zs
        for j in range(2, d + 1):
            Vc[:, j] = 2 * Zs * Vc[:, j - 1] - Vc[:, j - 2]
        A = np.vstack([Vc.real, Vc.imag])
        b = np.concatenate([F.real, F.imag])
        coef, *_ = np.linalg.lstsq(A, b, rcond=None)
        err = np.abs(Vc @ coef - F).max()
        if err < tol:
            return [float(c) for c in coef], lam_m, d
    return None


def _cheb_validate(x, er, ec, ev, coefs, lam_m, ncols=4, thresh=6e-3):
    """Host check: chebyshev recurrence vs Taylor reference on a few cols."""
    er = np.asarray(er).astype(np.int64)
    ec = np.asarray(ec).astype(np.int64)
    ev = np.asarray(ev).astype(np.float64)
    deg = np.zeros(N_NODES, np.float64)
    np.add.at(deg, er, ev)
    xs = np.asarray(x)[:, :ncols].astype(np.float64)

    def matvec(v):  # [N, c] -> L @ v
        out = np.empty_like(v)
        for c in range(v.shape[1]):
            wv = ev * v[ec, c]
            agg = np.bincount(er, weights=wv, minlength=N_NODES)
            out[:, c] = deg * v[:, c] - agg
        return out

    t_seg = DIFF_T / N_SEG
    y = xs.copy()
    for s in range(N_SEG):
        term = y.copy()
        acc = y.copy()
        for k in range(1, N_TERMS + 1):
            term = (-t_seg / k) * matvec(term)
            acc = acc + term
        y = acc

    d = len(coefs) - 1
    tprev = xs.copy()
    out = coefs[0] * xs
    tcur = (2.0 / lam_m) * matvec(xs) - xs   # A_hat @ x
    out = out + coefs[1] * tcur
    for j in range(2, d + 1):
        tnext = 2 * ((2.0 / lam_m) * matvec(tcur) - tcur) - tprev
        out = out + coefs[j] * tnext
        tprev, tcur = tcur, tnext
    rel = np.abs(out - y).max() / np.abs(y).max()
    return float(rel), rel < thresh


def _build(plan, nt=N_SEG * N_TERMS):
    import concourse.bacc as bacc
    import concourse.mybir as mybir
    from concourse.bass import AP

    batches = plan["batches"]
    runs = plan["runs"]
    QTOT = plan["QTOT"]
    n_ops = plan["n_ops"]
    NT = nt
    t_seg = DIFF_T / N_SEG

    nc = bacc.Bacc("TRN2", debug=False, num_devices=NCORES,
                   num_swdge_queues=1)
    x_in = nc.dram_tensor("x_region", [R, D_FEAT], mybir.dt.float32,
                          kind="ExternalInput").ap()
    gidx_in = nc.dram_tensor("gidx", [128, n_ops * 64], mybir.dt.int16,
                             kind="ExternalInput").ap()
    ab_in = nc.dram_tensor("ab", [128, QTOT * 2], mybir.dt.float16,
                           kind="ExternalInput").ap()
    out_t = nc.dram_tensor("out", [R, D_FEAT], mybir.dt.float32,
                           kind="ExternalOutput").ap()
    myreg = nc.dram_tensor("myreg", [R, D_FEAT], mybir.dt.float16,
                           kind="Internal").ap()
    tabs = [nc.dram_tensor(f"tab{i}", [NP, D_FEAT], mybir.dt.float16,
                           kind="Internal", addr_space="Shared").ap()
            for i in range(2)]

    x_pv = x_in.rearrange("(b p) f -> p b f", p=128)
    my_pv = myreg.rearrange("(b p) f -> p b f", p=128)
    out_pv = out_t.rearrange("(b p) f -> p b f", p=128)
    tab_gv = [t.rearrange("(a two) f -> a (two f)", two=2) for t in tabs]

    groups = [list(range(NCORES))]

    with (
        nc.sbuf_tensor([128, RAWQ * 512], mybir.dt.float16) as raw,
        nc.sbuf_tensor([128, RB * D_FEAT], mybir.dt.float32) as acc,
        nc.sbuf_tensor([128, RB * D_FEAT], mybir.dt.float32) as agg,
        nc.sbuf_tensor([128, n_ops * 64], mybir.dt.int16) as gidx,
        nc.sbuf_tensor([128, QTOT * 2], mybir.dt.float16) as ab,
        nc.semaphore() as usem,
        nc.semaphore() as gsem,
        nc.semaphore() as bsem,
        nc.semaphore() as vsem,
        nc.semaphore() as dsem,
        nc.semaphore() as csem,
        nc.Block() as block,
    ):
        n_batches = len(batches)
        rawap = raw[:]
        abap = ab[:]
        aggap = agg[:]
        acc3 = acc[:].rearrange("p (b f) -> p b f", f=D_FEAT)
        agg3 = agg[:].rearrange("p (b f) -> p b f", f=D_FEAT)

        @block.gpsimd
        def _(gpsimd):
            U = [0]

            def u(n=16):
                U[0] += n
                return U[0]

            gpsimd.dma_start(gidx[:], gidx_in[:]).then_inc(usem, 16); u()
            gpsimd.dma_start(ab[:], ab_in[:]).then_inc(usem, 16); u()
            gpsimd.dma_start(acc3[:, :, :], x_pv[:, :, :]).then_inc(usem, 16); u()
            gpsimd.memzero(agg[:]).then_inc(usem, 1); u(1)
            gpsimd.wait_ge(usem, U[0])

            DS = [0]
            CS = [0]
            # init: table <- x
            gpsimd.dma_start(my_pv[:, :, :], acc3[:, :, :]).then_inc(dsem, 16)
            DS[0] += 16
            gpsimd.wait_ge(dsem, DS[0])
            gpsimd.collective_compute(
                kind="AllGather", op=mybir.AluOpType.bypass,
                replica_groups=groups,
                ins=[myreg[:, :].opt()], outs=[tabs[1][:, :].opt()],
            ).then_inc(csem, 1)
            CS[0] += 1
            gpsimd.wait_ge(csem, CS[0])

            G = [0]
            B = [0]
            for t in range(1, NT + 1):
                gv = tab_gv[t % 2]
                for bt, (blks, q0, qp) in enumerate(batches):
                    gB = (t - 1) * n_batches + bt
                    if gB > 0:
                        gpsimd.wait_ge(bsem, gB)
                    ops_b = qp // QOP
                    o0 = q0 // QOP
                    for oi in range(ops_b):
                        o = o0 + oi
                        gpsimd.dma_gather(
                            out_ap=AP(rawap.tensor, rawap.offset + oi * QOP * 512,
                                      [rawap.ap[0], [512, QOP], [1, 512]]),
                            in_ap=gv[:, :],
                            idxs_ap=gidx[:, o * 64:(o + 1) * 64],
                            num_idxs=GIDX, num_idxs_reg=GIDX,
                            elem_size=512,
                            queue_num=0,
                        ).then_inc(gsem, 16)
                        G[0] += 16
                gpsimd.wait_ge(vsem, t)
                if t < NT:
                    src = acc3 if t % N_TERMS == 0 else agg3
                    gpsimd.dma_start(my_pv[:, :, :], src[:, :, :]).then_inc(dsem, 16)
                    DS[0] += 16
                    gpsimd.wait_ge(dsem, DS[0])
                    gpsimd.collective_compute(
                        kind="AllGather", op=mybir.AluOpType.bypass,
                        replica_groups=groups,
                        ins=[myreg[:, :].opt()],
                        outs=[tabs[(t + 1) % 2][:, :].opt()],
                    ).then_inc(csem, 1)
                    CS[0] += 1
                    gpsimd.wait_ge(csem, CS[0])

            gpsimd.wait_ge(vsem, NT)
            gpsimd.dma_start(out_pv[:, :, :], acc3[:, :, :]).then_inc(usem, 16); u()
            gpsimd.wait_ge(usem, U[0])

        @block.vector
        def _(vector):
            VG = [0]
            for t in range(1, NT + 1):
                k = (t - 1) % N_TERMS + 1
                s_t = float(t_seg / k)
                for bt, (blks, q0, qp) in enumerate(batches):
                    ops_b = qp // QOP
                    VG[0] += 16 * ops_b
                    vector.wait_ge(gsem, VG[0])
                    # raw *= ab  (parity-masked weights), in place
                    QB = qp
                    raw_ap = AP(rawap.tensor, rawap.offset,
                                [rawap.ap[0], [512, QB], [256, 2], [1, 256]])
                    ab_ap = AP(abap.tensor, abap.offset + q0 * 2,
                               [abap.ap[0], [2, QB], [1, 2], [0, 256]])
                    vector.tensor_tensor(raw_ap, raw_ap, ab_ap,
                                         op=mybir.AluOpType.mult)
                    for (qloc0, b0, nb, Sv) in runs[bt]:
                        in_ap = AP(rawap.tensor, rawap.offset + qloc0 * 512,
                                   [rawap.ap[0], [Sv * 512, nb], [1, 256],
                                    [512, Sv], [256, 2]])
                        out_ap = AP(aggap.tensor, aggap.offset + b0 * 256,
                                    [aggap.ap[0], [256, nb], [1, 256]])
                        inst = vector.tensor_reduce(
                            out_ap, in_ap, axis=mybir.AxisListType.XY,
                            op=mybir.AluOpType.add)
                    inst.then_inc(bsem, 1)
                vector.tensor_scalar_mul(agg[:], agg[:], s_t)
                vector.tensor_add(acc[:], acc[:], agg[:]).then_inc(vsem, 1)

    nc.compile()
    return nc


def _build_cheb(plan, coefs, lam_m, nt=None):
    """Chebyshev-recurrence variant: out = sum_j c_j T_j(A_hat) x,
    T_{j+1} = 2 A_hat T_j - T_{j-1}, A_hat = (2L - lam_m I)/lam_m.

    Device matvec computes agg = (W - D) u = -L u, so
    A_hat T = -(2/lam_m) agg - T."""
    import concourse.bacc as bacc
    import concourse.mybir as mybir
    from concourse.bass import AP

    batches = plan["batches"]
    runs = plan["runs"]
    QTOT = plan["QTOT"]
    n_ops = plan["n_ops"]
    d = len(coefs) - 1
    NT = d if nt is None else nt

    nc = bacc.Bacc("TRN2", debug=False, num_devices=NCORES,
                   num_swdge_queues=1)
    x_in = nc.dram_tensor("x_region", [R, D_FEAT], mybir.dt.float32,
                          kind="ExternalInput").ap()
    gidx_in = nc.dram_tensor("gidx", [128, n_ops * 64], mybir.dt.int16,
                             kind="ExternalInput").ap()
    ab_in = nc.dram_tensor("ab", [128, QTOT * 2], mybir.dt.float16,
                           kind="ExternalInput").ap()
    out_t = nc.dram_tensor("out", [R, D_FEAT], mybir.dt.float32,
                           kind="ExternalOutput").ap()
    myreg = nc.dram_tensor("myreg", [R, D_FEAT], mybir.dt.float16,
                           kind="Internal").ap()
    tabs = [nc.dram_tensor(f"tab{i}", [NP, D_FEAT], mybir.dt.float16,
                           kind="Internal", addr_space="Shared").ap()
            for i in range(2)]

    x_pv = x_in.rearrange("(b p) f -> p b f", p=128)
    my_pv = myreg.rearrange("(b p) f -> p b f", p=128)
    out_pv = out_t.rearrange("(b p) f -> p b f", p=128)
    tab_gv = [t.rearrange("(a two) f -> a (two f)", two=2) for t in tabs]
    groups = [list(range(NCORES))]
    FR = RB * D_FEAT  # 12544

    with (
        nc.sbuf_tensor([128, RAWQ * 512], mybir.dt.float16) as raw,
        nc.sbuf_tensor([128, FR], mybir.dt.float32) as agg,
        nc.sbuf_tensor([128, FR], mybir.dt.float16) as tb0,
        nc.sbuf_tensor([128, FR], mybir.dt.float16) as tb1,
        nc.sbuf_tensor([128, n_ops * 64], mybir.dt.int16) as gidx,
        nc.sbuf_tensor([128, QTOT * 2], mybir.dt.float16) as ab,
        nc.semaphore() as usem,
        nc.semaphore() as gsem,
        nc.semaphore() as bsem,
        nc.semaphore() as vsem,
        nc.semaphore() as dsem,
        nc.semaphore() as csem,
        nc.Block() as block,
    ):
        n_batches = len(batches)
        rawap = raw[:]
        abap = ab[:]
        aggap = agg[:]
        tb = [tb0, tb1]
        stage = rawap.bitcast(mybir.dt.float32)[:, 0:FR]
        stage3 = stage.rearrange("p (b f) -> p b f", f=D_FEAT)
        agg3 = aggap.rearrange("p (b f) -> p b f", f=D_FEAT)
        tb3 = [t[:].rearrange("p (b f) -> p b f", f=D_FEAT) for t in tb]

        def cj(j):
            return float(coefs[j]) if j <= d else 0.0

        def s1j(j):
            base = (-2.0 / lam_m) if j == 1 else (-4.0 / lam_m)
            return float(base if j <= d else base * 0.25)

        @block.gpsimd
        def _(gpsimd):
            U = [0]

            def u(n=16):
                U[0] += n
                return U[0]

            gpsimd.dma_start(gidx[:], gidx_in[:]).then_inc(usem, 16); u()
            gpsimd.dma_start(ab[:], ab_in[:]).then_inc(usem, 16); u()
            # T_0 = x (fp16 cast)
            gpsimd.dma_start(tb3[0][:, :, :], x_pv[:, :, :]).then_inc(
                usem, 16); u()
            gpsimd.memzero(agg[:]).then_inc(usem, 1); u(1)
            gpsimd.wait_ge(usem, U[0])
            gpsimd.engine_nop().then_inc(vsem, 1)  # unblock DVE init stage

            DS = [0]
            CS = [0]
            # table <- T_0; out <- c0 * T_0 (staged by DVE, vsem=2)
            gpsimd.dma_start(my_pv[:, :, :], tb3[0][:, :, :]).then_inc(
                dsem, 16)
            DS[0] += 16
            gpsimd.collective_compute(
                kind="AllGather", op=mybir.AluOpType.bypass,
                replica_groups=groups,
                ins=[myreg[:, :].opt()], outs=[tabs[1][:, :].opt()],
            ).then_inc(csem, 1)
            CS[0] += 1
            gpsimd.wait_ge(vsem, 2)
            gpsimd.dma_start(out_pv[:, :, :], stage3[:, :, :]).then_inc(
                usem, 16); u()
            gpsimd.wait_ge(dsem, DS[0])
            gpsimd.wait_ge(csem, CS[0])
            gpsimd.wait_ge(usem, U[0])

            for t in range(1, NT + 1):
                gv = tab_gv[t % 2]
                for bt, (blks, q0, qp) in enumerate(batches):
                    gB = (t - 1) * n_batches + bt
                    if gB > 0:
                        gpsimd.wait_ge(bsem, gB)
                    ops_b = qp // QOP
                    o0 = q0 // QOP
                    for oi in range(ops_b):
                        o = o0 + oi
                        gpsimd.dma_gather(
                            out_ap=AP(rawap.tensor,
                                      rawap.offset + oi * QOP * 512,
                                      [rawap.ap[0], [512, QOP], [1, 512]]),
                            in_ap=gv[:, :],
                            idxs_ap=gidx[:, o * 64:(o + 1) * 64],
                            num_idxs=GIDX, num_idxs_reg=GIDX,
                            elem_size=512,
                            queue_num=0,
                        ).then_inc(gsem, 16)
                gpsimd.wait_ge(vsem, t + 2)
                # out += c_t * T_t
                gpsimd.dma_start(out_pv[:, :, :], stage3[:, :, :],
                                 accum_op=mybir.AluOpType.add).then_inc(
                    dsem, 16)
                DS[0] += 16
                gpsimd.wait_ge(dsem, DS[0])
                if t < NT:
                    gpsimd.dma_start(my_pv[:, :, :],
                                     tb3[t % 2][:, :, :]).then_inc(dsem, 16)
                    DS[0] += 16
                    gpsimd.wait_ge(dsem, DS[0])
                    gpsimd.collective_compute(
                        kind="AllGather", op=mybir.AluOpType.bypass,
                        replica_groups=groups,
                        ins=[myreg[:, :].opt()],
                        outs=[tabs[(t + 1) % 2][:, :].opt()],
                    ).then_inc(csem, 1)
                    CS[0] += 1
                    gpsimd.wait_ge(csem, CS[0])

        @block.vector
        def _(vector):
            VG = [0]
            # init stage = c0 * T_0
            vector.wait_ge(vsem, 1)
            vector.tensor_scalar_mul(stage[:, :], tb[0][:], cj(0))
            vector.drain().then_inc(vsem, 1)
            for t in range(1, NT + 1):
                for bt, (blks, q0, qp) in enumerate(batches):
                    ops_b = qp // QOP
                    VG[0] += 16 * ops_b
                    vector.wait_ge(gsem, VG[0])
                    QB = qp
                    raw_ap = AP(rawap.tensor, rawap.offset,
                                [rawap.ap[0], [512, QB], [256, 2], [1, 256]])
                    ab_ap = AP(abap.tensor, abap.offset + q0 * 2,
                               [abap.ap[0], [2, QB], [1, 2], [0, 256]])
                    vector.tensor_tensor(raw_ap, raw_ap, ab_ap,
                                         op=mybir.AluOpType.mult)
                    for (qloc0, b0, nb, Sv) in runs[bt]:
                        in_ap = AP(rawap.tensor,
                                   rawap.offset + qloc0 * 512,
                                   [rawap.ap[0], [Sv * 512, nb], [1, 256],
                                    [512, Sv], [256, 2]])
                        out_ap = AP(aggap.tensor, aggap.offset + b0 * 256,
                                    [aggap.ap[0], [256, nb], [1, 256]])
                        inst = vector.tensor_reduce(
                            out_ap, in_ap, axis=mybir.AxisListType.XY,
                            op=mybir.AluOpType.add)
                    inst.then_inc(bsem, 1)
                # recurrence: agg = s1*agg - [T_{t-2}] - T_{t-1} (- T_{t-1})
                cur = (t + 1) % 2   # buffer holding T_{t-1}
                nxt = t % 2         # buffer holding T_{t-2}; becomes T_t
                vector.tensor_scalar_mul(agg[:], agg[:], s1j(t))
                if t >= 2:
                    vector.tensor_sub(agg[:], agg[:], tb[nxt][:])
                vector.tensor_sub(agg[:], agg[:], tb[cur][:])
                if t >= 2:
                    vector.tensor_sub(agg[:], agg[:], tb[cur][:])
                vector.tensor_copy(tb[nxt][:], agg[:])
                vector.tensor_scalar_mul(stage[:, :], agg[:], cj(t))
                vector.drain().then_inc(vsem, 1)

    nc.compile()
    return nc


def _get_compiled(plan, nt=N_SEG * N_TERMS):
    key = ("taylor", nt, plan["QTOT"],
           tuple(tuple(b[0]) + (b[1], b[2]) for b in plan["batches"]))
    if key not in _compiled:
        _compiled[key] = _build(plan, nt)
    return _compiled[key]


def _get_compiled_cheb(plan, coefs, lam_m, nt=None):
    key = ("cheb", nt, tuple(coefs), lam_m, plan["QTOT"],
           tuple(tuple(b[0]) + (b[1], b[2]) for b in plan["batches"]))
    if key not in _compiled:
        _compiled[key] = _build_cheb(plan, coefs, lam_m, nt)
    return _compiled[key]


def kernel(x, edge_rows, edge_cols, edge_vals):
    from concourse.bass_utils import run_bass_kernel_spmd

    plan, gidxs, abs_, x_regions, new_of_orig = _preprocess(
        x, edge_rows, edge_cols, edge_vals)

    nc = None
    fit = _cheb_fit(edge_rows, edge_cols, edge_vals)
    if fit is not None:
        coefs, lam_m, d = fit
        rel, ok = _cheb_validate(x, edge_rows, edge_cols, edge_vals,
                                 coefs, lam_m)
        if ok:
            nc = _get_compiled_cheb(plan, coefs, lam_m)
    if nc is None:
        nc = _get_compiled(plan)

    in_maps = []
    for c in range(NCORES):
        in_maps.append({
            "x_region": x_regions[c],
            "gidx": gidxs[c],
            "ab": abs_[c],
        })
    res = run_bass_kernel_spmd(nc, in_maps, core_ids=list(range(NCORES)))
    full = np.empty((NP, D_FEAT), np.float32)
    for c in range(NCORES):
        full[c * R:(c + 1) * R] = res.results[c]["out"]
    out = full[new_of_orig[:N_NODES]]
    return out.astype(np.float32)
